# revision 16
# baseline (speedup 1.0000x reference)
"""Trainium2 Bass kernel for Luong-attention (nn_Attention_4174708212176).

out[b] = softmax(dec[b] @ (enc[b] @ W)^T) @ enc[b],  b = 0..7, one batch per core.

v3 scheme — single-pass fp32r matmuls for M1/M2 (fp32 operands, ~2^-12.7
effective matmul precision at fp16 speed on TRN2), fp16 M3, DMA-XBAR
transpose, PSUM-direct softmax:
- M1 (ep = W^T @ encT): one fp32r pass, 64 matmuls. ep copied PSUM->SBUF in
  fp32 by ACT (8 x [128,1024] chunks) — no split/handoff pieces at all.
- M2 (logits = decT^T @ ep): one fp32r pass, 16 matmuls per t-tile
  (~3.9us/t-tile measured).
- Softmax on PSUM directly: DVE max per [128,1024] half, ACT Exp reads PSUM
  with accumulated row sums, P in fp16.
- P^T for M3 via one dma_start(transpose=True) per t-tile (DMA XBAR, off the
  PE). M3 in fp16. Output stored fp16 (host casts to fp32).

Measured HW per-matmul (N=512, rotating weights): fp16 277ns, fp32r 243ns,
fp8-DoubleRow 232ns; fp32r single-pass replaces the old fp16 hi/lo 3-pass
M1 and the fp16+fp8-cross M2 at better precision per cycle.

The PE stream is software-pipelined: transpose+M3 of tile t-1 are emitted
after tile t's logits so the PE never waits on the softmax engines.
"""
import contextlib
import numpy as np

import concourse.bass as bass
import concourse.tile as tile
from concourse import bacc, mybir
from concourse.bass_utils import run_bass_kernel_spmd

B, S, T, E, D = 8, 2048, 2048, 512, 512
P = 128
DO = D // P      # 4  d-tiles
EO = E // P      # 4  e-tiles
SO = S // P      # 16 s-tiles
TO = T // P      # 16 t-tiles
SC = S // 512    # 4  512-wide s-chunks
NCORES = 8

# fp32 input packing (free-dim offsets in fp32 elements per partition)
OFF_W = 0                     # W    [EO, D]
OFF_ET = OFF_W + EO * D       # encT [EO, S]
OFF_DT = OFF_ET + EO * S      # decT [DO, T]
FREE32 = OFF_DT + DO * T
FREE16 = SO * E               # enc natural fp16 [SO, E]

_compiled_nc = {}


def _build(reps=1):
    nc = bacc.Bacc()
    x_in = nc.declare_dram_parameter("x", [P, FREE16], mybir.dt.float16, isOutput=False)
    x8_in = nc.declare_dram_parameter("x8", [P, FREE32], mybir.dt.float32r, isOutput=False)
    out_d = nc.declare_dram_parameter("out", [T, E], mybir.dt.float16, isOutput=True)

    with tile.TileContext(nc) as tc:
        with tc.tile_pool(name="const", bufs=1) as cpool, \
             tc.tile_pool(name="ep", bufs=1) as eppool, \
             tc.tile_pool(name="work", bufs=3) as wpool, \
             tc.tile_pool(name="stat", bufs=4) as spool, \
             tc.tile_pool(name="psA", bufs=3, space="PSUM") as psA, \
             tc.tile_pool(name="psC", bufs=2, space="PSUM") as psC:

            _ENGS = (mybir.EngineType.PE, mybir.EngineType.Activation,
                     mybir.EngineType.DVE, mybir.EngineType.SP,
                     mybir.EngineType.Pool)
            loop_ctx = (tc.For_i(0, reps, 1, hint_engines=_ENGS)
                        if reps > 1 else contextlib.nullcontext())
            with loop_ctx:
                _body(nc, tc, cpool, eppool, wpool, spool, psA, psC,
                      x_in, x8_in, out_d)

    nc.compile()
    return nc


def _body(nc, tc, cpool, eppool, wpool, spool, psA, psC, x_in, x8_in, out_d):
    F32R = mybir.dt.float32r
    w32 = cpool.tile([P, EO, D], F32R, tag="w32", name="w32")
    e32 = cpool.tile([P, EO, S], F32R, tag="e32", name="e32")
    d32 = cpool.tile([P, DO, T], F32R, tag="d32", name="d32")
    en = cpool.tile([P, SO, E], mybir.dt.float16, tag="en", name="en")

    xap = x8_in.ap()
    nc.sync.dma_start(w32[:], xap[:, OFF_W:OFF_ET].rearrange(
        "p (a b) -> p a b", b=D))
    esrc = xap[:, OFF_ET:OFF_DT].rearrange("p (a b) -> p a b", b=S)
    # per-sc chunks so M1 can start after the first chunk lands
    for sc in range(SC):
        nc.sync.dma_start(e32[:, :, sc * 512:(sc + 1) * 512],
                          esrc[:, :, sc * 512:(sc + 1) * 512])
    dsrc = xap[:, OFF_DT:FREE32].rearrange("p (a b) -> p a b", b=T)
    # chunked along T so the next rep's refill overlaps this rep's tail
    for tc_ in range(4):
        nc.sync.dma_start(d32[:, :, tc_ * 512:(tc_ + 1) * 512],
                          dsrc[:, :, tc_ * 512:(tc_ + 1) * 512])
    nc.sync.dma_start(en[:], x_in.ap().rearrange("p (a b) -> p a b", b=E))

    def wt(eo, do):   # W tile [128, 128] (lhsT for M1)
        return w32[:, eo, do * P:(do + 1) * P]

    def et(eo, sc):   # encT chunk [128, 512] (rhs for M1)
        return e32[:, eo, sc * 512:(sc + 1) * 512]

    def dt_(do, tt):  # decT tile [128, 128] (lhsT for M2)
        return d32[:, do, tt * P:(tt + 1) * P]

    def encn(st):     # enc natural tile [128, 512] (rhs for M3)
        return en[:, st, :]

    # ---- M1: ep[d, s] = sum_e W[e, d] * encT[e, s], single fp32r pass.
    ep32 = eppool.tile([P, DO, S], F32R, tag="ep32", name="ep32")
    for half in range(2):
        for do in range(DO):
            ps = psA.tile([P, 1024], mybir.dt.float32, tag="ps",
                          name=f"m1_{half}_{do}")
            # eo-major: consecutive matmuls share the stationary W tile
            for eo in range(EO):
                for scl in range(2):
                    sc = 2 * half + scl
                    col = slice(scl * 512, (scl + 1) * 512)
                    nc.tensor.matmul(ps[:, col], wt(eo, do), et(eo, sc),
                                     start=(eo == 0), stop=(eo == EO - 1),
                                     skip_group_check=True)
            nc.scalar.copy(ep32[:, do, half * 1024:(half + 1) * 1024], ps[:])

    def ept(do, sc):  # ep chunk [128, 512] (rhs for M2)
        return ep32[:, do, sc * 512:(sc + 1) * 512]

    # ---- per t-tile: M2 logits -> softmax; transpose+M3 of the previous
    # tile are emitted after the next tile's M2 so the PE never waits on ACT.
    def emit_m2_softmax(tt):
        pss = []
        for half in range(2):
            ps = psA.tile([P, 1024], mybir.dt.float32, tag="ps",
                          name=f"m2_{tt}_{half}")
            pss.append(ps)
            # do-major: consecutive matmuls share the stationary dec tile
            for do in range(DO):
                for scl in range(2):
                    sc = 2 * half + scl
                    col = slice(scl * 512, (scl + 1) * 512)
                    nc.tensor.matmul(ps[:, col], dt_(do, tt), ept(do, sc),
                                     start=(do == 0), stop=(do == DO - 1),
                                     skip_group_check=True)

        # per-512 chunk maxes start as soon as each accumulation group stops
        pmax = spool.tile([P, SC], mybir.dt.float32, name=f"pmax{tt}", tag="pmax")
        for sc in range(SC):
            col = slice((sc % 2) * 512, (sc % 2) * 512 + 512)
            nc.vector.tensor_reduce(pmax[:, sc:sc + 1], pss[sc // 2][:, col],
                                    axis=mybir.AxisListType.X,
                                    op=mybir.AluOpType.max)
        negmax = spool.tile([P, 1], mybir.dt.float32, name=f"negmax{tt}",
                            tag="negmax")
        nc.vector.tensor_reduce(negmax[:], pmax[:], axis=mybir.AxisListType.X,
                                op=mybir.AluOpType.max, negate=True)

        p_sb = wpool.tile([P, S], mybir.dt.float16, name=f"p{tt}", tag="p")
        sums = spool.tile([P, 2], mybir.dt.float32, name=f"sums{tt}", tag="sums")
        for half in range(2):
            nc.scalar.activation(p_sb[:, half * 1024:(half + 1) * 1024],
                                 pss[half][:],
                                 mybir.ActivationFunctionType.Exp,
                                 bias=negmax[:], scale=1.0,
                                 accum_out=sums[:, half:half + 1])
        return p_sb, sums

    def emit_tr_m3(tt, p_sb, sums):
        # P [128t, 2048s] -> PT [128s, SO, 128t] via the DMA XBAR transpose,
        # split in halves so TR(h0) starts right after exp(h0) and M3's first
        # 8 s-tiles can begin while exp/TR of h1 are still in flight
        pt = wpool.tile([P, SO, P], mybir.dt.float16, name=f"pt{tt}", tag="pt")
        for half in range(2):
            nc.sync.dma_start(pt[:, half * 8:(half + 1) * 8, :],
                              p_sb[:, half * 1024:(half + 1) * 1024],
                              transpose=True)

        # M3: out[t, e] = sum_s PT[s, t]^T * enc_n[s, e]
        ops = psC.tile([P, E], mybir.dt.float32, tag="ps_out", name=f"m3_{tt}")
        for st in range(SO):
            nc.tensor.matmul(ops[:], pt[:, st, :], encn(st),
                             start=(st == 0), stop=(st == SO - 1))
        # ssum/recip emitted here (not with the exps) so they don't block the
        # next tile's pmax in the in-order DVE queue
        ssum = spool.tile([P, 1], mybir.dt.float32, name=f"ssum{tt}", tag="ssum")
        nc.vector.tensor_reduce(ssum[:], sums[:], axis=mybir.AxisListType.X,
                                op=mybir.AluOpType.add)
        recip = spool.tile([P, 1], mybir.dt.float32, name=f"recip{tt}", tag="recip")
        nc.vector.reciprocal(recip[:], ssum[:])
        out_sb = wpool.tile([P, E], mybir.dt.float16, name=f"o{tt}", tag="o")
        nc.scalar.activation(out_sb[:], ops[:],
                             mybir.ActivationFunctionType.Copy,
                             bias=0.0, scale=recip[:])
        nc.sync.dma_start(out_d.ap()[tt * P:(tt + 1) * P, :], out_sb[:])

    prev = None
    for tt in range(TO):
        cur = (tt, *emit_m2_softmax(tt))
        if prev is not None:
            emit_tr_m3(*prev)
        prev = cur
    emit_tr_m3(*prev)


def _part(x, ko):
    """[K, F] -> [128, ko, F], partition = k % 128."""
    return np.ascontiguousarray(x.reshape(ko, P, -1).transpose(1, 0, 2))


def _make_wseg(W):
    return _part(np.asarray(W, np.float32), EO).reshape(P, -1)


def _pack_core(enc_b, dec_b, wseg):
    encT = np.ascontiguousarray(enc_b.T)          # [512, 2048]
    decT = np.ascontiguousarray(dec_b.T)          # [512, 2048]
    x8 = np.concatenate([
        wseg,
        _part(encT, EO).reshape(P, -1),
        _part(decT, DO).reshape(P, -1),
    ], axis=1).astype(np.float32)
    x = _part(enc_b.astype(np.float16), SO).reshape(P, -1)
    return x, x8


def kernel(enc_hidden_states, dec_hidden_states, W_att):
    enc = np.asarray(enc_hidden_states, np.float32)
    dec = np.asarray(dec_hidden_states, np.float32)
    W = np.asarray(W_att, np.float32)

    wseg = _make_wseg(W)
    packed = [_pack_core(enc[b], dec[b], wseg) for b in range(NCORES)]
    in_maps = [{"x": p[0], "x8": p[1]} for p in packed]

    if 1 not in _compiled_nc:
        _compiled_nc[1] = _build(1)

    res = run_bass_kernel_spmd(_compiled_nc[1], in_maps, list(range(NCORES)))
    out = np.stack([res.results[b]["out"] for b in range(NCORES)], axis=0)
    return out.astype(np.float32)


if __name__ == "__main__":
    rng = np.random.default_rng(0)
    enc = rng.standard_normal((B, S, E), dtype=np.float32)
    dec = rng.standard_normal((B, T, D), dtype=np.float32)
    W = rng.standard_normal((E, D), dtype=np.float32)
    out = kernel(enc, dec, W)
    print("out", out.shape, out.dtype)


# revision 20
# speedup vs baseline: 1.1013x; 1.1013x over previous
"""Trainium2 Bass kernel for Luong-attention (nn_Attention_4174708212176).

out[b] = softmax(dec[b] @ (enc[b] @ W)^T) @ enc[b],  b = 0..7, one batch per core.

v3 scheme — single-pass fp32r matmuls for M1/M2 (fp32 operands, ~2^-12.7
effective matmul precision at fp16 speed on TRN2), fp16 M3, DMA-XBAR
transpose, PSUM-direct softmax:
- M1 (ep = W^T @ encT): one fp32r pass, 64 matmuls. ep copied PSUM->SBUF in
  fp32 by ACT (8 x [128,1024] chunks) — no split/handoff pieces at all.
- M2 (logits = decT^T @ ep): one fp32r pass, 16 matmuls per t-tile
  (~3.9us/t-tile measured).
- Softmax on PSUM directly: DVE max per 512-chunk (starts as each
  accumulation group stops), ACT Exp reads PSUM with accumulated row sums,
  P in fp16.
- P^T for M3 via dma_start(transpose=True) per half t-tile (DMA XBAR, off
  the PE); M3's first 8 s-tiles start while the second half transposes.
  M3 in fp16. Output stored fp16 (host casts to fp32).

Measured HW per-matmul (N=512, rotating weights): fp16 277ns, fp32r 243ns,
fp8-DoubleRow 232ns; fp32r single-pass replaces the old fp16 hi/lo 3-pass
M1 and the fp16+fp8-cross M2 at better precision per cycle.

The PE stream is software-pipelined: transpose+M3 of tile t-1 are emitted
after tile t's logits so the PE never waits on the softmax engines.
"""
import contextlib
import numpy as np

import concourse.bass as bass
import concourse.tile as tile
from concourse import bacc, mybir
from concourse.bass_utils import run_bass_kernel_spmd

B, S, T, E, D = 8, 2048, 2048, 512, 512
P = 128
DO = D // P      # 4  d-tiles
EO = E // P      # 4  e-tiles
SO = S // P      # 16 s-tiles
TO = T // P      # 16 t-tiles
SC = S // 512    # 4  512-wide s-chunks
NCORES = 8

# fp32 input packing (free-dim offsets in fp32 elements per partition)
OFF_W = 0                     # W    [EO, D]
OFF_ET = OFF_W + EO * D       # encT [EO, S]
OFF_DT = OFF_ET + EO * S      # decT [DO, T]
FREE32 = OFF_DT + DO * T
FREE16 = SO * E               # enc natural fp16 [SO, E]

_compiled_nc = {}


def _build(reps=1):
    nc = bacc.Bacc()
    x_in = nc.declare_dram_parameter("x", [P, FREE16], mybir.dt.float16, isOutput=False)
    x8_in = nc.declare_dram_parameter("x8", [P, FREE32], mybir.dt.float32r, isOutput=False)
    out_d = nc.declare_dram_parameter("out", [T, E], mybir.dt.float16, isOutput=True)

    with tile.TileContext(nc) as tc:
        with tc.tile_pool(name="const", bufs=1) as cpool, \
             tc.tile_pool(name="ep", bufs=1) as eppool, \
             tc.tile_pool(name="work", bufs=3) as wpool, \
             tc.tile_pool(name="stat", bufs=4) as spool, \
             tc.tile_pool(name="psA", bufs=3, space="PSUM") as psA, \
             tc.tile_pool(name="psC", bufs=2, space="PSUM") as psC:

            # Pool has no instructions in the body; omitting it drops a
            # useless per-iteration loop barrier on an idle engine
            _ENGS = (mybir.EngineType.PE, mybir.EngineType.Activation,
                     mybir.EngineType.DVE, mybir.EngineType.SP)
            loop_ctx = (tc.For_i(0, reps, 1, hint_engines=_ENGS)
                        if reps > 1 else contextlib.nullcontext())
            with loop_ctx:
                _body(nc, tc, cpool, eppool, wpool, spool, psA, psC,
                      x_in, x8_in, out_d)

    nc.compile()
    return nc


def _body(nc, tc, cpool, eppool, wpool, spool, psA, psC, x_in, x8_in, out_d):
    F32R = mybir.dt.float32r
    w32 = cpool.tile([P, EO, D], F32R, tag="w32", name="w32")
    e32 = cpool.tile([P, EO, S], F32R, tag="e32", name="e32")
    d32 = cpool.tile([P, DO, T], F32R, tag="d32", name="d32")
    en = cpool.tile([P, SO, E], mybir.dt.float16, tag="en", name="en")

    xap = x8_in.ap()
    nc.sync.dma_start(w32[:], xap[:, OFF_W:OFF_ET].rearrange(
        "p (a b) -> p a b", b=D))
    esrc = xap[:, OFF_ET:OFF_DT].rearrange("p (a b) -> p a b", b=S)
    # per-sc chunks so M1 can start after the first chunk lands
    for sc in range(SC):
        nc.sync.dma_start(e32[:, :, sc * 512:(sc + 1) * 512],
                          esrc[:, :, sc * 512:(sc + 1) * 512])
    dsrc = xap[:, OFF_DT:FREE32].rearrange("p (a b) -> p a b", b=T)
    # chunked along T so the next rep's refill overlaps this rep's tail
    for tc_ in range(4):
        nc.sync.dma_start(d32[:, :, tc_ * 512:(tc_ + 1) * 512],
                          dsrc[:, :, tc_ * 512:(tc_ + 1) * 512])
    nc.sync.dma_start(en[:], x_in.ap().rearrange("p (a b) -> p a b", b=E))

    def wt(eo, do):   # W tile [128, 128] (lhsT for M1)
        return w32[:, eo, do * P:(do + 1) * P]

    def et(eo, sc):   # encT chunk [128, 512] (rhs for M1)
        return e32[:, eo, sc * 512:(sc + 1) * 512]

    def dt_(do, tt):  # decT tile [128, 128] (lhsT for M2)
        return d32[:, do, tt * P:(tt + 1) * P]

    def encn(st):     # enc natural tile [128, 512] (rhs for M3)
        return en[:, st, :]

    # ---- M1: ep[d, s] = sum_e W[e, d] * encT[e, s], single fp32r pass.
    ep32 = eppool.tile([P, DO, S], F32R, tag="ep32", name="ep32")
    for half in range(2):
        for do in range(DO):
            ps = psA.tile([P, 1024], mybir.dt.float32, tag="ps",
                          name=f"m1_{half}_{do}")
            # eo-major: consecutive matmuls share the stationary W tile
            for eo in range(EO):
                for scl in range(2):
                    sc = 2 * half + scl
                    col = slice(scl * 512, (scl + 1) * 512)
                    nc.tensor.matmul(ps[:, col], wt(eo, do), et(eo, sc),
                                     start=(eo == 0), stop=(eo == EO - 1),
                                     skip_group_check=True)
            nc.scalar.copy(ep32[:, do, half * 1024:(half + 1) * 1024], ps[:])

    def ept(do, sc):  # ep chunk [128, 512] (rhs for M2)
        return ep32[:, do, sc * 512:(sc + 1) * 512]

    # ---- per t-tile: M2 logits -> softmax; transpose+M3 of the previous
    # tile are emitted after the next tile's M2 so the PE never waits on ACT.
    def emit_m2_softmax(tt):
        pss = []
        for half in range(2):
            ps = psA.tile([P, 1024], mybir.dt.float32, tag="ps",
                          name=f"m2_{tt}_{half}")
            pss.append(ps)
            # do-major: consecutive matmuls share the stationary dec tile
            for do in range(DO):
                for scl in range(2):
                    sc = 2 * half + scl
                    col = slice(scl * 512, (scl + 1) * 512)
                    nc.tensor.matmul(ps[:, col], dt_(do, tt), ept(do, sc),
                                     start=(do == 0), stop=(do == DO - 1),
                                     skip_group_check=True)

        # per-512 chunk maxes start as soon as each accumulation group stops
        pmax = spool.tile([P, SC], mybir.dt.float32, name=f"pmax{tt}", tag="pmax")
        for sc in range(SC):
            col = slice((sc % 2) * 512, (sc % 2) * 512 + 512)
            nc.vector.tensor_reduce(pmax[:, sc:sc + 1], pss[sc // 2][:, col],
                                    axis=mybir.AxisListType.X,
                                    op=mybir.AluOpType.max)
        negmax = spool.tile([P, 1], mybir.dt.float32, name=f"negmax{tt}",
                            tag="negmax")
        nc.vector.tensor_reduce(negmax[:], pmax[:], axis=mybir.AxisListType.X,
                                op=mybir.AluOpType.max, negate=True)

        p_sb = wpool.tile([P, S], mybir.dt.float16, name=f"p{tt}", tag="p")
        sums = spool.tile([P, 2], mybir.dt.float32, name=f"sums{tt}", tag="sums")
        for half in range(2):
            nc.scalar.activation(p_sb[:, half * 1024:(half + 1) * 1024],
                                 pss[half][:],
                                 mybir.ActivationFunctionType.Exp,
                                 bias=negmax[:], scale=1.0,
                                 accum_out=sums[:, half:half + 1])
        return p_sb, sums

    def emit_tr_m3(tt, p_sb, sums):
        # P [128t, 2048s] -> PT [128s, SO, 128t] via the DMA XBAR transpose,
        # split in halves so TR(h0) starts right after exp(h0) and M3's first
        # 8 s-tiles can begin while exp/TR of h1 are still in flight
        pt = wpool.tile([P, SO, P], mybir.dt.float16, name=f"pt{tt}", tag="pt")
        for half in range(2):
            nc.sync.dma_start(pt[:, half * 8:(half + 1) * 8, :],
                              p_sb[:, half * 1024:(half + 1) * 1024],
                              transpose=True)

        # M3: out[t, e] = sum_s PT[s, t]^T * enc_n[s, e]
        ops = psC.tile([P, E], mybir.dt.float32, tag="ps_out", name=f"m3_{tt}")
        for st in range(SO):
            nc.tensor.matmul(ops[:], pt[:, st, :], encn(st),
                             start=(st == 0), stop=(st == SO - 1))
        # ssum/recip emitted here (not with the exps) so they don't block the
        # next tile's pmax in the in-order DVE queue
        ssum = spool.tile([P, 1], mybir.dt.float32, name=f"ssum{tt}", tag="ssum")
        nc.vector.tensor_reduce(ssum[:], sums[:], axis=mybir.AxisListType.X,
                                op=mybir.AluOpType.add)
        recip = spool.tile([P, 1], mybir.dt.float32, name=f"recip{tt}", tag="recip")
        nc.vector.reciprocal(recip[:], ssum[:])
        out_sb = wpool.tile([P, E], mybir.dt.float16, name=f"o{tt}", tag="o")
        nc.scalar.activation(out_sb[:], ops[:],
                             mybir.ActivationFunctionType.Copy,
                             bias=0.0, scale=recip[:])
        nc.sync.dma_start(out_d.ap()[tt * P:(tt + 1) * P, :], out_sb[:])

    prev = None
    for tt in range(TO):
        cur = (tt, *emit_m2_softmax(tt))
        if prev is not None:
            emit_tr_m3(*prev)
        prev = cur
    emit_tr_m3(*prev)


def _part(x, ko):
    """[K, F] -> [128, ko, F], partition = k % 128."""
    return np.ascontiguousarray(x.reshape(ko, P, -1).transpose(1, 0, 2))


def _make_wseg(W):
    return _part(np.asarray(W, np.float32), EO).reshape(P, -1)


def _pack_core(enc_b, dec_b, wseg):
    encT = np.ascontiguousarray(enc_b.T)          # [512, 2048]
    decT = np.ascontiguousarray(dec_b.T)          # [512, 2048]
    x8 = np.concatenate([
        wseg,
        _part(encT, EO).reshape(P, -1),
        _part(decT, DO).reshape(P, -1),
    ], axis=1).astype(np.float32)
    x = _part(enc_b.astype(np.float16), SO).reshape(P, -1)
    return x, x8


def kernel(enc_hidden_states, dec_hidden_states, W_att):
    enc = np.asarray(enc_hidden_states, np.float32)
    dec = np.asarray(dec_hidden_states, np.float32)
    W = np.asarray(W_att, np.float32)

    wseg = _make_wseg(W)
    packed = [_pack_core(enc[b], dec[b], wseg) for b in range(NCORES)]
    in_maps = [{"x": p[0], "x8": p[1]} for p in packed]

    if 1 not in _compiled_nc:
        _compiled_nc[1] = _build(1)

    res = run_bass_kernel_spmd(_compiled_nc[1], in_maps, list(range(NCORES)))
    out = np.stack([res.results[b]["out"] for b in range(NCORES)], axis=0)
    return out.astype(np.float32)


if __name__ == "__main__":
    rng = np.random.default_rng(0)
    enc = rng.standard_normal((B, S, E), dtype=np.float32)
    dec = rng.standard_normal((B, T, D), dtype=np.float32)
    W = rng.standard_normal((E, D), dtype=np.float32)
    out = kernel(enc, dec, W)
    print("out", out.shape, out.dtype)
